# revision 19
# baseline (speedup 1.0000x reference)
"""Causal self-attention (B=2, T=2048, C=1024, H=16, D=64) on 8 trn2 NeuronCores.

v3 redesign vs v2 (282us). Sharding unchanged (batch x head-group; core c:
batch c//4, head pair-group c%4). Trace findings on v2: true PE occupancy
was only 66% (two 26-31us stalls where proj MMs interleaved as filler waited
on late AllGathers and blocked the FIFO PE queue), the attention inner loop
was ACT-bound (exp 1.15us/stage vs 640ns of PE work), the normalize chain
burned 50us of DVE/gpsimd on single-partition copies + partition_broadcast,
and the first MM started 17us in (cold DMA head) with HAM oscillating all
kernel long.

Changes:
  - Software-pipelined QKV<->attention: attention for query-chunk qt starts
    as soon as K(<=qt), V(<=qt), Q(qt) are computed; the remaining QKV MMs
    are pumped 1-8 at a time between attention stages so the PE always has
    ~1.1us of work per stage while ACT does the exp. ACT's ~85us of exp is
    fully hidden under the PE stream instead of serializing after QKV.
  - Softmax denominator via 64 ones-columns in the V tile (M=128): the
    denominator lands replicated on PSUM partitions 64-127, so normalize is
    just reciprocal_approx_fast([64,512]) + tensor_mul - no [1,512] copies,
    no gpsimd partition_broadcast.
  - Per-(qt,pair) AllGathers (8 x 128KB) triggered immediately after each
    pair's normalize; proj(qt0,qt1) run as filler only inside qt3's
    attention (their gathers completed ~2 qt earlier - no stall risk);
    proj(qt2) + proj(qt3) at the end, with proj(qt3) contracting the
    pair0-half first to hide the last gather's flight time.
  - 22 dummy warm MMs at t~0.5us flip the HAM clock gate to 8/8 during the
    initial x DMA; host pre-arranges x/weights so every load is a
    contiguous 4KB-per-partition DMA (first real MM ~5us instead of 17).
  - Bias adds skipped when biases are zero (they are in this problem).

Host: bf16 casts, transposes, W_proj row permute, final concat (free).
"""

import os
import numpy as np
import ml_dtypes

import concourse.bass as bass
import concourse.bacc as bacc
import concourse.mybir as mybir
import concourse.tile as tile
from concourse import bass_utils
from concourse.bass import ds, ts
from concourse.bass_interp import get_hw_module

P = 128
B, T, C = 2, 2048, 1024
NH, D = 16, 64
NC = 8          # cores
NG = 4          # head groups (cores per batch)
HL = NH // NG   # heads per core = 4
DL = HL * D     # local channels = 256
NQ = 512        # query chunk
NQT = T // NQ   # 4 query chunks
F32 = mybir.dt.float32
BF16 = mybir.dt.bfloat16
GROUPS = [[0, 1, 2, 3], [4, 5, 6, 7]]
N_WARM_MM = 22


def _build_body(ctx, tc, io, has_bias):
    nc = tc.nc
    (xt, wq, wk, wv, bq, bk, bv, wp, bp, tri, out,
     warm_in, warm_out, ytl, ytf, ytl3, ytf3) = io
    mm = nc.tensor.matmul

    pers = ctx.enter_context(tc.tile_pool(name="pers", bufs=1))

    # --- warmups: ACT exp table load + collective barrier/ncfw wakeup ---
    warm = pers.tile([1, 16], F32)
    nc.vector.memset(warm[:], 0.0)
    nc.scalar.activation(warm[:], warm[:], mybir.ActivationFunctionType.Exp,
                         bias=0.0, scale=1.0)
    nc.gpsimd.collective_compute(
        "AllGather", mybir.AluOpType.bypass, replica_groups=GROUPS,
        ins=[warm_in[:]], outs=[warm_out[:]])

    # --- PSUM pools: s 2x[128,2,512] = 4 banks; o 2x[128,512] = 2; qp 2 = 2
    ps_pool = ctx.enter_context(tc.tile_pool(name="ps", bufs=1, space="PSUM"))
    sbp = ctx.enter_context(tc.tile_pool(name="sbp", bufs=3))
    nrm = ctx.enter_context(tc.tile_pool(name="nrm", bufs=2))
    outp = ctx.enter_context(tc.tile_pool(name="outp", bufs=2))

    # --- HAM warm: dummy matmuls on a zeroed tile, run during the x DMA ---
    warm_sb = pers.tile([P, P], BF16)
    nc.vector.memset(warm_sb[:], 0.0)
    warm_ps = ps_pool.tile([P, NQ], F32, tag="qp", bufs=2, name="warm_ps")
    for i in range(N_WARM_MM):
        mm(warm_ps[:, 0:P], warm_sb[:], warm_sb[:],
           start=(i == 0), stop=(i == N_WARM_MM - 1))

    # --- persistent SBUF ---
    tri_sb = pers.tile([P, P], BF16)
    xt_sb = pers.tile([P, NQT, C // P, NQ], BF16)   # [p, tq, o, t]
    wq_sb = pers.tile([P, C // P, DL], BF16)
    wk_sb = pers.tile([P, C // P, DL], BF16)
    wv_sb = pers.tile([P, C // P, DL], BF16)
    wp_sb = pers.tile([P, C // P, DL], BF16)
    qt_sb = pers.tile([P, 2, T], BF16)   # [j_part, pair, t]
    kt_sb = pers.tile([P, 2, T], BF16)
    v_sb = pers.tile([P, T // P, HL, 2 * D], BF16)  # [t_part, lc, head, d|ones]
    yth = [pers.tile([P, T], BF16, tag=f"yth{p}", name=f"yth{p}") for p in range(2)]
    # gathered Y^T: combined (both pairs) for qt0-2, per-pair for qt3
    ytfc_sb = [pers.tile([P, 2 * NG, NQ], BF16, tag=f"ytfc{i}", name=f"ytfc{i}")
               for i in range(3)]
    ytf3_sb = [pers.tile([P, NG, NQ], BF16, tag=f"ytf3{i}", name=f"ytf3{i}")
               for i in range(2)]

    # ones block of V via contiguous memset + strided copy (rank-4 strided
    # memset is a sim-vs-HW divergence risk)
    ones_st = pers.tile([P, (T // P) * HL * D], BF16)
    nc.vector.memset(ones_st[:], 1.0)
    nc.vector.tensor_copy(
        v_sb[:, :, :, D:2 * D],
        ones_st[:].rearrange("p (a h d) -> p a h d", a=T // P, h=HL))

    # DMA order: what the first matmuls need comes first. x0 is split in two
    # so the first K matmuls start after half the chunk lands.
    nc.sync.dma_start(wk_sb[:], wk)
    nc.sync.dma_start(xt_sb[:, 0, 0:4], xt[0][:, 0:4])
    nc.sync.dma_start(xt_sb[:, 0, 4:8], xt[0][:, 4:8])
    nc.sync.dma_start(wv_sb[:], wv)
    nc.sync.dma_start(wq_sb[:], wq)
    nc.sync.dma_start(tri_sb[:], tri)
    nc.sync.dma_start(xt_sb[:, 1], xt[1])
    nc.sync.dma_start(xt_sb[:, 2], xt[2])
    nc.sync.dma_start(xt_sb[:, 3], xt[3])
    nc.sync.dma_start(wp_sb[:], wp)

    if has_bias:
        bqp = pers.tile([P, 2], F32)
        nc.sync.dma_start(bqp[:], bq.rearrange("(j p) -> p j", p=P))
        bkp = pers.tile([P, 2], F32)
        nc.sync.dma_start(bkp[:], bk.rearrange("(j p) -> p j", p=P))
        bpp = pers.tile([P, 2], F32)
        nc.sync.dma_start(bpp[:], bp.rearrange("(j p) -> p j", p=P))
        bv_row = pers.tile([1, DL], F32)
        nc.sync.dma_start(bv_row[:], bv[None, :])
        bv_bc = pers.tile([P, DL], F32)
        nc.gpsimd.partition_broadcast(bv_bc[:], bv_row[:])

    # ---------------- QKV work units (generators; one yield per MM) -------
    # NOTE: every unit yields exactly once per MM, and the trailing drain is
    # emitted BEFORE the final yield — so "pumped >= 8*k" guarantees the k-th
    # unit's drain (and the data it publishes to SBUF) is already emitted.
    def qk_unit(w_sb, b_sb, dst, j, tq):
        """one [128,512] tile of Q^T or K^T: 8 accumulating MMs + drain."""
        ps = ps_pool.tile([P, NQ], F32, tag="qp", bufs=2, name="qk_ps")
        for cc in range(C // P):
            mm(ps[:], w_sb[:, cc, ts(j, P)], xt_sb[:, tq, cc, :],
               start=(cc == 0), stop=(cc == C // P - 1))
            if cc < C // P - 1:
                yield
        if has_bias:
            nc.vector.tensor_scalar_add(
                dst[:, j, ts(tq, NQ)], ps[:], b_sb[:, j:j + 1])
        else:
            nc.vector.tensor_copy(dst[:, j, ts(tq, NQ)], ps[:])
        yield

    def v_unit(tt):
        """V chunk tt: [128 t, 256 dl] -> v_sb[:, tt, :, 0:64]."""
        ps = ps_pool.tile([P, NQ], F32, tag="qp", bufs=2, name="v_ps")
        tq, sub = divmod(tt, 4)
        for cc in range(C // P):
            mm(ps[:, 0:DL], xt_sb[:, tq, cc, ds(sub * P, P)], wv_sb[:, cc, :],
               start=(cc == 0), stop=(cc == C // P - 1))
            if cc < C // P - 1:
                yield
        src = ps[:, 0:DL].rearrange("p (h d) -> p h d", h=HL)
        if has_bias:
            nc.vector.tensor_add(
                v_sb[:, tt, :, 0:D], src,
                bv_bc[:].rearrange("p (h d) -> p h d", h=HL))
        else:
            nc.vector.tensor_copy(v_sb[:, tt, :, 0:D], src)
        yield

    def proj_unit(qt):
        """out^T[:, qt chunk] for qt<=2: 2 j-tiles x 8 MMs over combined ytf
        (rows are in natural W_proj order: o-chunk = 2*g + pair)."""
        o_sb = outp.tile([P, 2, NQ], BF16, tag="o_sb", bufs=2, name="o_sb")
        for j in range(2):
            ps = ps_pool.tile([P, NQ], F32, tag="qp", bufs=2, name="pj_ps")
            for oc in range(C // P):
                mm(ps[:], wp_sb[:, oc, ts(j, P)], ytfc_sb[qt][:, oc, :],
                   start=(oc == 0), stop=(oc == C // P - 1))
                if oc < C // P - 1:
                    yield
            if has_bias:
                nc.vector.tensor_scalar_add(o_sb[:, j, :], ps[:], bpp[:, j:j + 1])
            else:
                nc.vector.tensor_copy(o_sb[:, j, :], ps[:])
            if j == 1:
                nc.gpsimd.dma_start(
                    out[:, ts(qt, NQ)].rearrange("(a p) t -> p a t", p=P),
                    o_sb[:])
            yield

    def proj_unit_tail(qt):
        """qt3 proj over the two per-pair gathers, pair0 half (both j) first,
        so the pair1 gather's flight hides behind the pair0 MMs. Per-pair
        gather rows: chunk cc = core cc's pair-p channels = wp o-chunk
        2*cc + pair."""
        o_sb = outp.tile([P, 2, NQ], BF16, tag="o_sb", bufs=2, name="o_sb")
        pss = [ps_pool.tile([P, NQ], F32, tag="qp", bufs=2, name=f"pjt{j}")
               for j in range(2)]
        warm_ps2 = ps_pool.tile([P, NQ], F32, tag="o", bufs=2, name="warm_ps2")

        def drain(j):
            if has_bias:
                nc.vector.tensor_scalar_add(o_sb[:, j, :], pss[j][:], bpp[:, j:j + 1])
            else:
                nc.vector.tensor_copy(o_sb[:, j, :], pss[j][:])

        for pair in range(2):
            for j in range(2):
                for cc in range(NG):
                    mm(pss[j][:], wp_sb[:, 2 * cc + pair, ts(j, P)],
                       ytf3_sb[pair][:, cc, :],
                       start=(pair == 0 and cc == 0),
                       stop=(pair == 1 and cc == NG - 1))
                    if not (pair == 1 and j == 1 and cc == NG - 1):
                        yield
                if pair == 1 and j == 0:
                    drain(0)  # overlaps the final j1 MMs
            if pair == 0:
                # keep the PE busy (and the HAM clock gate warm) through the
                # pair1 gather's flight; these run only while pv1 data is late
                for i in range(40):
                    mm(warm_ps2[:, 0:P], warm_sb[:], warm_sb[:],
                       start=(i == 0), stop=(i == 39))
        drain(1)
        nc.gpsimd.dma_start(
            out[:, ts(qt, NQ)].rearrange("(a p) t -> p a t", p=P), o_sb[:])
        yield

    # ---------------- filler pump ----------------
    filler = []
    state = {"pumped": 0}

    def pump(k):
        done = 0
        while filler and done < k:
            try:
                next(filler[0])
                done += 1
                state["pumped"] += 1
            except StopIteration:
                filler.pop(0)

    def pump_until(target):
        while filler and state["pumped"] < target:
            pump(1)

    for tq in range(NQT):
        filler.append(qk_unit(wk_sb, bkp if has_bias else None, kt_sb, 0, tq))
        filler.append(qk_unit(wk_sb, bkp if has_bias else None, kt_sb, 1, tq))
        for tt in range(4 * tq, 4 * tq + 4):
            filler.append(v_unit(tt))
        filler.append(qk_unit(wq_sb, bqp if has_bias else None, qt_sb, 0, tq))
        filler.append(qk_unit(wq_sb, bqp if has_bias else None, qt_sb, 1, tq))

    # ---------------- attention ----------------
    def attn_qt(qt, quota):
        """attention for query chunk qt; pumps `quota` filler MMs spread
        across the 2*nl stages."""
        q0 = NQ * qt
        nl = q0 // P + NQ // P
        nstages = 2 * nl
        # front-load the filler (cap per stage) so the last prereq unit's
        # DVE drain lands well before the qt boundary -- otherwise it
        # head-of-line-blocks the normalize that frees the o PSUM buffers
        cap = max(2, min(8, -(-quota // max(nstages - 6, 1))))
        pumps = []
        left = quota
        for _ in range(nstages):
            k = min(cap, left)
            pumps.append(k)
            left -= k
        si = 0

        for pair in range(2):
            o_ps = [
                ps_pool.tile([P, NQ], F32, tag="o", bufs=2, name=f"o_ps{hi}")
                for hi in range(2)
            ]

            def s_stage(lc):
                w0 = max(P * lc - q0, 0)
                s2 = ps_pool.tile([P, 2, NQ], F32, tag="s", bufs=2, name="s_ps")
                for hi in range(2):
                    mm(s2[:, hi, w0:NQ],
                       kt_sb[ds(64 * hi, 64), pair, ts(lc, P)],
                       qt_sb[ds(64 * hi, 64), pair, ds(q0 + w0, NQ - w0)],
                       start=True, stop=True, tile_position=(64 * hi, 0))
                return s2

            def pv_stage(lc, s2):
                off = P * lc - q0
                w0 = max(off, 0)
                pt = sbp.tile([P, 2, NQ], BF16, tag="pt", name="pt")
                nc.scalar.activation(
                    pt[:, :, w0:NQ], s2[:, :, w0:NQ],
                    mybir.ActivationFunctionType.Exp,
                    bias=0.0, scale=float(1.0 / np.sqrt(D)))
                if off >= 0:
                    for hi in range(2):
                        nc.vector.tensor_mul(
                            pt[:, hi, ds(off, P)], pt[:, hi, ds(off, P)],
                            tri_sb[:])
                for hi in range(2):
                    mm(o_ps[hi][:, w0:NQ],
                       v_sb[:, lc, 2 * pair + hi, :],
                       pt[:, hi, w0:NQ],
                       start=(lc == 0), stop=(lc == nl - 1))

            prev = s_stage(0)
            for lc in range(1, nl):
                cur = s_stage(lc)
                pump(pumps[si]); si += 1
                pv_stage(lc - 1, prev)
                prev = cur
            pump(pumps[si]); si += 1
            pv_stage(nl - 1, prev)

            # normalize: denominator is replicated on partitions 64-127.
            # (copy PSUM->SBUF first: reciprocal_approx_fast reading PSUM
            # directly returns ~5% wrong values on HW, sim disagrees)
            for hi in range(2):
                den = nrm.tile([D, NQ], F32, tag="den", bufs=2, name="den")
                nc.vector.tensor_copy(den[:], o_ps[hi][ds(D, D), :])
                rcp = nrm.tile([D, NQ], F32, tag="rcp", bufs=2, name="rcp")
                nc.vector.reciprocal_approx_fast(rcp[:], den[:])
                nc.vector.tensor_mul(
                    yth[pair][ds(64 * hi, 64), ds(q0, NQ)],
                    o_ps[hi][0:D, :], rcp[:])
                # store each head-half as soon as its normalize lands, so the
                # first store overlaps the second mul
                dst = ytl[qt][ds(pair * P + 64 * hi, 64), :] if qt < 3 \
                    else ytl3[pair][ds(64 * hi, 64), :]
                nc.gpsimd.dma_start(dst, yth[pair][ds(64 * hi, 64), ds(q0, NQ)])
            if qt < 3:
                if pair == 1:
                    nc.gpsimd.collective_compute(
                        "AllGather", mybir.AluOpType.bypass,
                        replica_groups=GROUPS,
                        ins=[ytl[qt][:]], outs=[ytf[qt][:]])
                    # split the load so proj MMs can start after each half
                    src = ytf[qt].rearrange("(o p) t -> p o t", p=P)
                    nc.sync.dma_start(ytfc_sb[qt][:, 0:4], src[:, 0:4])
                    nc.sync.dma_start(ytfc_sb[qt][:, 4:8], src[:, 4:8])
            else:
                nc.gpsimd.collective_compute(
                    "AllGather", mybir.AluOpType.bypass, replica_groups=GROUPS,
                    ins=[ytl3[pair][:]], outs=[ytf3[pair][:]])
                src = ytf3[pair].rearrange("(o p) t -> p o t", p=P)
                nc.sync.dma_start(ytf3_sb[pair][:, 0:2], src[:, 0:2])
                nc.sync.dma_start(ytf3_sb[pair][:, 2:4], src[:, 2:4])

    for qt in range(NQT):
        pump_until(64 * (qt + 1))
        if qt == NQT - 1:
            filler.append(proj_unit(0))
            filler.append(proj_unit(1))
            quota = 32
        else:
            quota = 64
        attn_qt(qt, quota)
    pump(1 << 30)

    for _ in proj_unit(2):
        pass
    for _ in proj_unit_tail(3):
        pass


def build_program(has_bias):
    nc = bacc.Bacc(
        "TRN2",
        target_bir_lowering=False,
        debug=False,
        enable_asserts=False,
        num_devices=NC,
    )
    xt = nc.dram_tensor("xt", [NQT, P, C // P, NQ], BF16, kind="ExternalInput").ap()
    wq = nc.dram_tensor("wq", [P, C // P, DL], BF16, kind="ExternalInput").ap()
    wk = nc.dram_tensor("wk", [P, C // P, DL], BF16, kind="ExternalInput").ap()
    wv = nc.dram_tensor("wv", [P, C // P, DL], BF16, kind="ExternalInput").ap()
    wp = nc.dram_tensor("wp", [P, C // P, DL], BF16, kind="ExternalInput").ap()
    bq = nc.dram_tensor("bq", [DL], F32, kind="ExternalInput").ap()
    bk = nc.dram_tensor("bk", [DL], F32, kind="ExternalInput").ap()
    bv = nc.dram_tensor("bv", [DL], F32, kind="ExternalInput").ap()
    bp = nc.dram_tensor("bp", [DL], F32, kind="ExternalInput").ap()
    tri = nc.dram_tensor("tri", [P, P], BF16, kind="ExternalInput").ap()
    out = nc.dram_tensor("out", [DL, T], BF16, kind="ExternalOutput").ap()
    warm_in = nc.dram_tensor("warm_in", [1, 16], F32, kind="Internal").ap()
    warm_out = nc.dram_tensor("warm_out", [4, 16], F32, kind="Internal").ap()
    ytl = [
        nc.dram_tensor(f"ytl{i}", [2 * P, NQ], BF16, kind="Internal").ap()
        for i in range(3)
    ]
    ytf = [
        nc.dram_tensor(f"ytf{i}", [NG * 2 * P, NQ], BF16, kind="Internal").ap()
        for i in range(3)
    ]
    ytl3 = [
        nc.dram_tensor(f"ytl3{i}", [P, NQ], BF16, kind="Internal").ap()
        for i in range(2)
    ]
    ytf3 = [
        nc.dram_tensor(f"ytf3{i}", [NG * P, NQ], BF16, kind="Internal").ap()
        for i in range(2)
    ]
    io = (xt, wq, wk, wv, bq, bk, bv, wp, bp, tri, out,
          warm_in, warm_out, ytl, ytf, ytl3, ytf3)
    with tile.TileContext(nc) as tc:
        import contextlib

        with contextlib.ExitStack() as ctx:
            _build_body(ctx, tc, io, has_bias)
    nc.compile()
    return nc


def make_in_maps(x, W_attn, b_attn, W_proj, b_proj):
    tri_np = np.triu(np.ones((P, P), dtype=np.float32)).astype(ml_dtypes.bfloat16)
    x = np.asarray(x, dtype=np.float32)
    W_attn = np.asarray(W_attn, dtype=np.float32)
    b_attn = np.asarray(b_attn, dtype=np.float32)
    W_proj = np.asarray(W_proj, dtype=np.float32)
    b_proj = np.asarray(b_proj, dtype=np.float32)
    bf = ml_dtypes.bfloat16

    def wlayout(w):  # [C, DL] -> [P, C//P, DL], row c = o*P + p
        return np.ascontiguousarray(
            w.reshape(C // P, P, DL).transpose(1, 0, 2)).astype(bf)

    in_maps = []
    for c in range(NC):
        b, g = divmod(c, NG)
        cols = slice(DL * g, DL * (g + 1))
        # x[b].T is [C, T]; chunk into [tq][P, C//P, NQ]
        xT = x[b].T
        xt_arr = np.empty((NQT, P, C // P, NQ), dtype=bf)
        for tq in range(NQT):
            xt_arr[tq] = (
                xT[:, tq * NQ:(tq + 1) * NQ]
                .reshape(C // P, P, NQ).transpose(1, 0, 2).astype(bf)
            )
        in_maps.append(
            {
                "xt": xt_arr,
                "wq": wlayout(W_attn[:, cols]),
                "wk": wlayout(W_attn[:, C:][:, cols]),
                "wv": wlayout(W_attn[:, 2 * C:][:, cols]),
                "wp": wlayout(W_proj[:, cols]),
                "bq": np.ascontiguousarray(b_attn[cols]),
                "bk": np.ascontiguousarray(b_attn[C:][cols]),
                "bv": np.ascontiguousarray(b_attn[2 * C:][cols]),
                "bp": np.ascontiguousarray(b_proj[cols]),
                "tri": tri_np,
            }
        )
    return in_maps


_NC_CACHE = {}


def _install_ntff_hook():
    import sys
    import types

    if "antenv.axon_hooks" in sys.modules:
        return True
    try:
        from trn_agent_boot.trn_boot import _ntff_profile_via_ctypes

        hook = _ntff_profile_via_ctypes("/opt/axon/libaxon_pjrt.so")
        if hook is None:
            return False
        mod = types.ModuleType("antenv.axon_hooks")
        mod.get_axon_ntff_profile_hook = lambda: hook
        mod.set_axon_ntff_profile_hook = lambda h: None
        sys.modules["antenv.axon_hooks"] = mod
        import antenv

        antenv.axon_hooks = mod
        bass_utils.upload_artifacts = lambda tmpdir: tmpdir
        return True
    except Exception:
        return False


def _get_program(has_bias):
    key = ("nc", has_bias)
    if key not in _NC_CACHE:
        nc = build_program(has_bias)
        nc.m = get_hw_module(nc.m)
        _NC_CACHE[key] = nc
    return _NC_CACHE[key]


def kernel(x, W_attn, b_attn, W_proj, b_proj):
    has_bias = bool(np.any(np.asarray(b_attn)) or np.any(np.asarray(b_proj)))
    nc = _get_program(has_bias)
    in_maps = make_in_maps(x, W_attn, b_attn, W_proj, b_proj)
    trace = bool(int(os.environ.get("KERNEL_TRACE", "0")))
    if trace:
        trace = _install_ntff_hook()
    res = bass_utils.run_bass_kernel_spmd(
        nc,
        in_maps,
        core_ids=list(range(NC)),
        trace=trace,
        trace_cores=list(range(NC)) if trace else None,
    )
    if trace:
        _NC_CACHE["last_results"] = res
        if res.exec_time_ns is not None:
            print(f"HW exec time: {res.exec_time_ns} ns")
            if res.instructions_and_trace is not None:
                print(f"trace: {res.instructions_and_trace[1]}")
    out = np.empty((B, T, C), dtype=np.float32)
    for c in range(NC):
        b, g = divmod(c, NG)
        out[b, :, DL * g:DL * (g + 1)] = (
            res.results[c]["out"].astype(np.float32).T
        )
    return out
